# revision 2
# baseline (speedup 1.0000x reference)
"""Multi-head causal attention (B=4, S=2048, D=1024, 16 heads) on 8 TRN2 cores.

Sharding: core c -> (batch b = c//2, head-group g = c%2). Each core computes
8 heads of one batch element end-to-end (QKV proj, causal softmax attention,
out-proj rows for its head slice). Host sums the two head-group partials per
batch and adds the output bias.

Per-core pipeline (all matmuls contraction-on-partitions, bf16 in / f32 psum):
  QT/KT[dtile] = (x @ w)^T   [128p = 2 heads x 64, S]
  Vones[kb]    = [V | 1]     [128p = k, h, 65]
  attention per (512-wide q-chunk, head-pair); the pair's score matmuls are
  packed into PE row groups via tile_position; two k-blocks share one
  [128,1024] score psum so each exp covers ~1024 cols; PV matmuls trail two
  iterations behind so PE never waits on the exp:
    ST[k,q] = KT.T @ QT; PT = exp(ST/8) bf16; tri-mask on diagonal 128 cols
    ctx[q 128, h, 65] += PT.T @ [V_h | 1]  (PT slice is the lhsT: output has
      q on partitions -> full 128-row PE output, ~half the PV stream cols of
      the [65, q] orientation; col 64 = softmax denominator)
    nq[q, d] = ctx * (1/denominator)  per-partition scale on DVE
    cxtq[d, q] = DMA-xbar-transpose(nq)  per (head-pair, 128-q tile)
  out[seq128, 512] = cxtq.T @ ow, streamed to DRAM per q-chunk; the final
  q-chunk's out-proj borrows the freed score psum banks.
"""

import numpy as np
import ml_dtypes

B, S, D = 4, 2048, 1024
H_TOT = 16
HD = 64
NCORES = 8
GH = 8          # heads per core
GD = GH * HD    # 512: dout slice per core
NKB = S // 128  # 16 k-blocks
NQC = S // 512  # 4 q-chunks
BF16 = ml_dtypes.bfloat16

PACK_HEADS = True   # pack 2 heads' score matmuls into PE row groups

_cache = {}


def _build_body(tc, nc, mybir, xT, wq, wk, wv, ow, outp):
    from concourse.masks import make_upper_triangular
    import contextlib

    dt = mybir.dt
    F = mybir.ActivationFunctionType

    pools = contextlib.ExitStack()
    tc_pool = lambda **kw: pools.enter_context(tc.tile_pool(**kw))

    singles = tc_pool(name="singles", bufs=1)
    pt_pool = tc_pool(name="pt", bufs=8)
    small = tc_pool(name="small", bufs=4)
    nq_pool = tc_pool(name="nq", bufs=8)
    ost_pool = tc_pool(name="ost", bufs=5)
    psum_st = tc_pool(name="psum_st", bufs=2, space="PSUM")
    psum_ctx = tc_pool(name="psum_ctx", bufs=3, space="PSUM")
    psum_mm = tc_pool(name="psum_mm", bufs=1, space="PSUM")

    # ---- persistent SBUF tensors (split per producer/consumer region so the
    # dependency tracker never over-serializes) ----
    xT_sb = [singles.tile([128, S], dt.bfloat16, name=f"xt{t}")
             for t in range(8)]
    wq_sb = [singles.tile([128, GD], dt.bfloat16, name=f"wq{t}")
             for t in range(8)]
    wk_sb = [singles.tile([128, GD], dt.bfloat16, name=f"wk{t}")
             for t in range(8)]
    wv_sb = [singles.tile([128, GD], dt.bfloat16, name=f"wv{t}")
             for t in range(8)]
    ow_sb = [singles.tile([128, D], dt.bfloat16, name=f"ow{t}")
             for t in range(4)]
    qt_sb = [singles.tile([128, S], dt.bfloat16, name=f"qt{t}")
             for t in range(4)]                              # 2 heads / dtile
    kt_sb = [singles.tile([128, S], dt.bfloat16, name=f"kt{t}")
             for t in range(4)]
    vo_sb = [singles.tile([128, GH, 65], dt.bfloat16, name=f"vo{t}")
             for t in range(NKB)]                            # [V_h | ones]
    # normalized ctx^T tiles [128 = 2 heads x 64 d, 128 q] per (pair, seq128)
    cxtq_sb = {(hp, sq): singles.tile([128, 128], dt.bfloat16,
                                      name=f"cq{hp}_{sq}")
               for hp in range(4) for sq in range(NKB)}
    tri = singles.tile([128, 128], dt.bfloat16)              # keep k<=q

    make_upper_triangular(nc, tri, val=1.0, diag=True)
    for t in range(NKB):
        nc.vector.memset(vo_sb[t][:, :, 64:65], 1.0)

    # ---- input DMAs (split per 128-row tile; first matmuls need wq t0 + xT t0) ----
    xT_r = xT.ap().rearrange("(t p) s -> p t s", p=128)
    wq_r = wq.ap().rearrange("(t p) n -> p t n", p=128)
    wk_r = wk.ap().rearrange("(t p) n -> p t n", p=128)
    wv_r = wv.ap().rearrange("(t p) n -> p t n", p=128)
    ow_r = ow.ap().rearrange("(t p) n -> p t n", p=128)
    DIN_ORDER = list(range(8))
    for t in range(8):
        nc.sync.dma_start(out=xT_sb[t], in_=xT_r[:, t, :])
        nc.sync.dma_start(out=wq_sb[t], in_=wq_r[:, t, :])
        if t % 2 == 1:
            nc.sync.dma_start(out=wk_sb[t // 2], in_=wk_r[:, t // 2, :])
    for t in range(4, 8):
        nc.sync.dma_start(out=wk_sb[t], in_=wk_r[:, t, :])
    for t in range(8):
        nc.sync.dma_start(out=wv_sb[t], in_=wv_r[:, t, :])
    for t in range(4):
        nc.sync.dma_start(out=ow_sb[t], in_=ow_r[:, t, :])

    def emit_proj_dtile(w_sb, t_sb, dtile):
        # din-outer: tolerate in-flight xT DMAs; 2 stp slots = 4 psum halves
        pst = [psum_st.tile([128, 1024], dt.float32, name="stp")
               for _ in range(2)]
        pss = [pst[0][:, 0:512], pst[0][:, 512:1024],
               pst[1][:, 0:512], pst[1][:, 512:1024]]
        for i, din in enumerate(DIN_ORDER):
            for c in range(4):
                nc.tensor.matmul(
                    pss[c],
                    lhsT=w_sb[din][:, dtile * 128:(dtile + 1) * 128],
                    rhs=xT_sb[din][:, c * 512:(c + 1) * 512],
                    start=(i == 0),
                    stop=(i == 7),
                )
        for c in range(4):
            nc.vector.tensor_copy(
                out=t_sb[dtile][:, c * 512:(c + 1) * 512], in_=pss[c])

    def emit_v(st):
        ps = psum_mm.tile([128, 512], dt.float32, name="mmps")
        for din in range(8):
            nc.tensor.matmul(
                ps,
                lhsT=xT_sb[din][:, st * 128:(st + 1) * 128],
                rhs=wv_sb[din],
                start=(din == 0),
                stop=(din == 7),
            )
        nc.vector.tensor_copy(
            out=vo_sb[st][:, :, 0:64],
            in_=ps.rearrange("p (h d) -> p h d", h=GH),
        )

    def emit_attn_pair(qc, hp):
        """512-wide q chunk qc for heads h0=2*hp (array rows 0:64) and
        h1=2*hp+1 (rows 64:128); scores packed into PE row groups.

        Two k-blocks share one [128,1024] ST psum tile so each exp covers up
        to 1024 columns. PV matmuls (PT as lhsT, [V|1] as rhs, output
        [128 q, 65]) trail two iterations behind so the PE never sits
        directly behind the exp on the ACT engine."""
        nkb = 4 * qc + 4
        q0 = 512 * qc
        # per half: [128 q, 4 qsb x 65] ctx accumulator in ONE psum bank.
        # Zero-region/group semantics: start=True only on the very first
        # matmul into the bank (zeroes the whole 2KB region; sibling qsb
        # slices consume the pending-zero with start=False), stop=True only
        # on the very last.
        ctxs = [psum_ctx.tile([128, 512], dt.float32, name="ctx")
                for _ in range(2)]
        started = [False, False]
        pend2 = []

        def emit_pv(kbs, offs, ns, pts):
            for half in range(2):
                for (kb, off, n) in zip(kbs, offs, ns):
                    j0 = (512 - n) // 128
                    for j in range(j0, 4):
                        c0 = off + (j - j0) * 128
                        nc.tensor.matmul(
                            ctxs[half][:, j * 65:j * 65 + 65],
                            lhsT=pts[half][:, c0:c0 + 128],
                            rhs=vo_sb[kb][:, 2 * hp + half, :],
                            start=not started[half],
                            stop=(kb == nkb - 1 and j == 3),
                        )
                        started[half] = True

        for kb0 in range(0, nkb, 2):
            kbs = [kb for kb in (kb0, kb0 + 1) if kb < nkb]
            ns = [512 - max(0, kb * 128 - q0) for kb in kbs]
            offs = [0] + [ns[0]] * (len(kbs) - 1)
            pts = []
            for half in range(2):
                p0 = half * 64
                stp = psum_st.tile([128, 1024], dt.float32, name="stp")
                for kb, off, n in zip(kbs, offs, ns):
                    nc.tensor.matmul(
                        stp[:, off:off + n],
                        lhsT=kt_sb[hp][p0:p0 + 64, kb * 128:(kb + 1) * 128],
                        rhs=qt_sb[hp][p0:p0 + 64, q0 + 512 - n:q0 + 512],
                        start=True,
                        stop=True,
                        tile_position=(p0, 0) if PACK_HEADS else None,
                    )
                ntot = offs[-1] + ns[-1]
                pt = pt_pool.tile([128, 1024], dt.bfloat16, name="pt")
                nc.scalar.activation(
                    out=pt[:, :ntot], in_=stp[:, :ntot], func=F.Exp,
                    scale=0.125)
                for kb, off in zip(kbs, offs):
                    if kb >= 4 * qc:  # diagonal: mask first 128 cols
                        nc.vector.tensor_mul(
                            pt[:, off:off + 128], pt[:, off:off + 128], tri)
                pts.append(pt)
            pend2.append((kbs, offs, ns, pts))
            if len(pend2) > 2:
                emit_pv(*pend2.pop(0))
        for p in pend2:
            emit_pv(*p)
        # normalize: per-partition scale by 1/denominator, then xbar-transpose
        # each [128 q, 128 d]-pair tile into cxtq (d on partitions).
        nqs = [nq_pool.tile([128, 128], dt.bfloat16, name="nq")
               for _ in range(4)]
        for half in range(2):
            ctx_t = ctxs[half]
            # [128, 4] AP of the 4 denominator columns (stride 65, offset 64)
            dens = ctx_t[:, 0:260].rearrange("p (j c) -> p c j", c=65)[:, 64, :]
            recip = small.tile([128, 4], dt.float32, name="recip")
            nc.vector.reciprocal(out=recip, in_=dens)
            for j in range(4):
                nc.vector.tensor_scalar_mul(
                    nqs[j][:, half * 64:half * 64 + 64],
                    ctx_t[:, j * 65:j * 65 + 64],
                    recip[:, j:j + 1],
                )
        for j in range(4):
            nc.sync.dma_start_transpose(
                out=cxtq_sb[(hp, 4 * qc + j)], in_=nqs[j])

    def emit_p4(sq_lo, sq_hi, final=False):
        for sq in range(sq_lo, sq_hi):
            for oc in range(2):
                if final:  # attention done: rotate over ALL freed banks
                    k = (sq * 2 + oc) % 3
                    if k == 0:
                        ps = psum_st.tile([128, 1024], dt.float32,
                                          name="stp")[:, 0:512]
                    elif k == 1:
                        ps = psum_ctx.tile([128, 512], dt.float32, name="ctx")
                    else:
                        ps = psum_mm.tile([128, 512], dt.float32, name="mmps")
                else:
                    ps = psum_mm.tile([128, 512], dt.float32, name="mmps")
                for dvt in range(4):
                    nc.tensor.matmul(
                        ps,
                        lhsT=cxtq_sb[(dvt, sq)],
                        rhs=ow_sb[dvt][:, oc * 512:(oc + 1) * 512],
                        start=(dvt == 0),
                        stop=(dvt == 3),
                    )
                ost = ost_pool.tile([128, 512], dt.float32, name="ost")
                nc.vector.tensor_copy(out=ost, in_=ps)
                nc.sync.dma_start(
                    out=outp.ap()[sq * 128:(sq + 1) * 128,
                                  oc * 512:(oc + 1) * 512],
                    in_=ost,
                )

    # ---- emission schedule: pipeline projections with qc=0 attention ----
    v_ranges = [range(0, 4), range(4, 8), range(8, 12), range(12, 16)]
    for dtile in range(4):
        emit_proj_dtile(wq_sb, qt_sb, dtile)
        emit_proj_dtile(wk_sb, kt_sb, dtile)
        for st in v_ranges[dtile]:
            emit_v(st)
        emit_attn_pair(0, dtile)
    for qc in range(1, NQC):
        for hp in range(4):
            emit_attn_pair(qc, hp)
            if hp == 0:
                emit_p4(4 * (qc - 1), 4 * qc)
    emit_p4(12, 16, final=True)

    return pools


def _build_nc():
    import concourse.tile as tile
    from concourse import bacc, mybir

    dt = mybir.dt
    nc = bacc.Bacc("TRN2", target_bir_lowering=False, debug=False,
                   num_devices=NCORES)
    xT = nc.dram_tensor("xt", [D, S], dt.bfloat16, kind="ExternalInput")
    wq = nc.dram_tensor("wq", [D, GD], dt.bfloat16, kind="ExternalInput")
    wk = nc.dram_tensor("wk", [D, GD], dt.bfloat16, kind="ExternalInput")
    wv = nc.dram_tensor("wv", [D, GD], dt.bfloat16, kind="ExternalInput")
    ow = nc.dram_tensor("ow", [GD, D], dt.bfloat16, kind="ExternalInput")
    outp = nc.dram_tensor("outp", [S, D], dt.float32, kind="ExternalOutput")

    with tile.TileContext(nc) as tc:
        pools = _build_body(tc, nc, mybir, xT, wq, wk, wv, ow, outp)
        pools.close()
    nc.compile()
    return nc


LAST_RESULTS = None


def kernel(batch, w_query, w_key, w_value, out_w, out_b):
    global LAST_RESULTS
    import os
    from concourse import bass_utils

    try:  # BASS_TRACE needs the axon NTFF hook; without it the run crashes
        from antenv.axon_hooks import get_axon_ntff_profile_hook  # noqa: F401
    except ImportError:
        os.environ.setdefault("BASS_NEVER_TRACE", "1")

    batch = np.asarray(batch, dtype=np.float32)
    w_query = np.asarray(w_query, dtype=np.float32)
    w_key = np.asarray(w_key, dtype=np.float32)
    w_value = np.asarray(w_value, dtype=np.float32)
    out_w = np.asarray(out_w, dtype=np.float32)
    out_b = np.asarray(out_b, dtype=np.float32)

    if "nc" not in _cache:
        _cache["nc"] = _build_nc()
    nc = _cache["nc"]

    xts = [np.ascontiguousarray(batch[b].T).astype(BF16) for b in range(B)]
    slc = [slice(g * GD, (g + 1) * GD) for g in range(2)]
    wqs = [np.ascontiguousarray(w_query[:, s]).astype(BF16) for s in slc]
    wks = [np.ascontiguousarray(w_key[:, s]).astype(BF16) for s in slc]
    wvs = [np.ascontiguousarray(w_value[:, s]).astype(BF16) for s in slc]
    ows = [np.ascontiguousarray(out_w[s, :]).astype(BF16) for s in slc]
    in_maps = []
    for c in range(NCORES):
        b, g = divmod(c, 2)
        in_maps.append({
            "xt": xts[b], "wq": wqs[g], "wk": wks[g],
            "wv": wvs[g], "ow": ows[g],
        })

    res = bass_utils.run_bass_kernel_spmd(
        nc, in_maps, core_ids=list(range(NCORES)),
    )
    LAST_RESULTS = res

    out = np.empty((B, S, D), np.float32)
    for b in range(B):
        out[b] = res.results[2 * b]["outp"] + res.results[2 * b + 1]["outp"] \
            + out_b[None, :]
    return out


# revision 6
# speedup vs baseline: 1.0018x; 1.0018x over previous
"""Multi-head causal attention (B=4, S=2048, D=1024, 16 heads) on 8 TRN2 cores.

Sharding: core c -> (batch b = c//2, head-group g = c%2). Each core computes
8 heads of one batch element end-to-end (QKV proj, causal softmax attention,
out-proj rows for its head slice). Host sums the two head-group partials per
batch and adds the output bias.

Per-core pipeline (all matmuls contraction-on-partitions, bf16 in / f32 psum):
  QT/KT[dtile] = (x @ w)^T   [128p = 2 heads x 64, S]
  Vones[kb]    = [V | 1]     [128p = k, h, 65]
  attention per (512-wide q-chunk, head-pair); the pair's score matmuls are
  packed into PE row groups via tile_position; two k-blocks share one
  [128,1024] score psum so each exp covers ~1024 cols; PV matmuls trail two
  iterations behind so PE never waits on the exp:
    ST[k,q] = KT.T @ QT; PT = exp(ST/8) bf16; tri-mask on diagonal 128 cols
    ctx[q 128, h, 65] += PT.T @ [V_h | 1]  (PT slice is the lhsT: output has
      q on partitions -> full 128-row PE output, ~half the PV stream cols of
      the [65, q] orientation; col 64 = softmax denominator)
    nq[q, d] = ctx * (1/denominator)  per-partition scale on DVE
    cxtq[d, q] = DMA-xbar-transpose(nq)  per (head-pair, 128-q tile)
  out[seq128, 512] = cxtq.T @ ow, streamed to DRAM per q-chunk; the final
  q-chunk's out-proj borrows the freed score psum banks.
"""

import numpy as np
import ml_dtypes

B, S, D = 4, 2048, 1024
H_TOT = 16
HD = 64
NCORES = 8
GH = 8          # heads per core
GD = GH * HD    # 512: dout slice per core
NKB = S // 128  # 16 k-blocks
NQC = S // 512  # 4 q-chunks
BF16 = ml_dtypes.bfloat16

PACK_HEADS = True   # pack 2 heads' score matmuls into PE row groups

_cache = {}


def _build_body(tc, nc, mybir, xT, wq, wk, wv, ow, outp):
    from concourse.masks import make_identity, make_lower_triangular
    import contextlib

    dt = mybir.dt
    F = mybir.ActivationFunctionType

    pools = contextlib.ExitStack()
    tc_pool = lambda **kw: pools.enter_context(tc.tile_pool(**kw))

    singles = tc_pool(name="singles", bufs=1)
    pt_pool = tc_pool(name="pt", bufs=8)
    small = tc_pool(name="small", bufs=4)
    nq_pool = tc_pool(name="nq", bufs=8)
    ost_pool = tc_pool(name="ost", bufs=5)
    psum_st = tc_pool(name="psum_st", bufs=2, space="PSUM")
    psum_ctx = tc_pool(name="psum_ctx", bufs=3, space="PSUM")
    psum_mm = tc_pool(name="psum_mm", bufs=1, space="PSUM")

    # ---- persistent SBUF tensors (split per producer/consumer region so the
    # dependency tracker never over-serializes) ----
    xT_sb = [singles.tile([128, S], dt.bfloat16, name=f"xt{t}")
             for t in range(8)]
    wq_sb = [singles.tile([128, GD], dt.bfloat16, name=f"wq{t}")
             for t in range(8)]
    wk_sb = [singles.tile([128, GD], dt.bfloat16, name=f"wk{t}")
             for t in range(8)]
    wv_sb = [singles.tile([128, GD], dt.bfloat16, name=f"wv{t}")
             for t in range(8)]
    ow_sb = [singles.tile([128, D], dt.bfloat16, name=f"ow{t}")
             for t in range(4)]
    qt_sb = [singles.tile([128, S], dt.bfloat16, name=f"qt{t}")
             for t in range(4)]                              # 2 heads / dtile
    kt_sb = [singles.tile([128, S], dt.bfloat16, name=f"kt{t}")
             for t in range(4)]
    vo_sb = [singles.tile([128, GH, 65], dt.bfloat16, name=f"vo{t}")
             for t in range(NKB)]                            # [V_h | ones]
    # normalized ctx^T tiles [128 = 2 heads x 64 d, 128 q] per (pair, seq128)
    cxtq_sb = {(hp, sq): singles.tile([128, 128], dt.bfloat16,
                                      name=f"cq{hp}_{sq}")
               for hp in range(4) for sq in range(NKB)}
    # causal mask folded into the score matmul: one extra accumulating
    # matmul per diagonal 128-col block adds -1e9 where k > q, so the exp
    # zeroes them and the DVE is off the exp->PV critical path entirely.
    ident = singles.tile([128, 128], dt.bfloat16)
    msk = singles.tile([128, 128], dt.bfloat16)              # -1e9 at k > q

    make_identity(nc, ident)
    make_lower_triangular(nc, msk, val=-1e9, diag=False)
    for t in range(NKB):
        nc.vector.memset(vo_sb[t][:, :, 64:65], 1.0)

    # ---- input DMAs (split per 128-row tile; first matmuls need wq t0 + xT t0) ----
    xT_r = xT.ap().rearrange("(t p) s -> p t s", p=128)
    wq_r = wq.ap().rearrange("(t p) n -> p t n", p=128)
    wk_r = wk.ap().rearrange("(t p) n -> p t n", p=128)
    wv_r = wv.ap().rearrange("(t p) n -> p t n", p=128)
    ow_r = ow.ap().rearrange("(t p) n -> p t n", p=128)
    DIN_ORDER = list(range(8))
    for t in range(8):
        nc.sync.dma_start(out=xT_sb[t], in_=xT_r[:, t, :])
        nc.sync.dma_start(out=wq_sb[t], in_=wq_r[:, t, :])
        if t % 2 == 1:
            nc.sync.dma_start(out=wk_sb[t // 2], in_=wk_r[:, t // 2, :])
    for t in range(4, 8):
        nc.sync.dma_start(out=wk_sb[t], in_=wk_r[:, t, :])
    for t in range(8):
        nc.sync.dma_start(out=wv_sb[t], in_=wv_r[:, t, :])
    for t in range(4):
        nc.sync.dma_start(out=ow_sb[t], in_=ow_r[:, t, :])

    def emit_proj_dtile(w_sb, t_sb, dtile):
        # din-outer: tolerate in-flight xT DMAs; 2 stp slots = 4 psum halves
        pst = [psum_st.tile([128, 1024], dt.float32, name="stp")
               for _ in range(2)]
        pss = [pst[0][:, 0:512], pst[0][:, 512:1024],
               pst[1][:, 0:512], pst[1][:, 512:1024]]
        for i, din in enumerate(DIN_ORDER):
            for c in range(4):
                nc.tensor.matmul(
                    pss[c],
                    lhsT=w_sb[din][:, dtile * 128:(dtile + 1) * 128],
                    rhs=xT_sb[din][:, c * 512:(c + 1) * 512],
                    start=(i == 0),
                    stop=(i == 7),
                )
        for c in range(4):
            nc.vector.tensor_copy(
                out=t_sb[dtile][:, c * 512:(c + 1) * 512], in_=pss[c])

    def emit_v(st):
        ps = psum_mm.tile([128, 512], dt.float32, name="mmps")
        for din in range(8):
            nc.tensor.matmul(
                ps,
                lhsT=xT_sb[din][:, st * 128:(st + 1) * 128],
                rhs=wv_sb[din],
                start=(din == 0),
                stop=(din == 7),
            )
        nc.vector.tensor_copy(
            out=vo_sb[st][:, :, 0:64],
            in_=ps.rearrange("p (h d) -> p h d", h=GH),
        )

    def emit_attn_pair(qc, hp):
        """512-wide q chunk qc for heads h0=2*hp (array rows 0:64) and
        h1=2*hp+1 (rows 64:128); scores packed into PE row groups.

        Two k-blocks share one [128,1024] ST psum tile so each exp covers up
        to 1024 columns. PV matmuls (PT as lhsT, [V|1] as rhs, output
        [128 q, 65]) trail two iterations behind so the PE never sits
        directly behind the exp on the ACT engine."""
        nkb = 4 * qc + 4
        q0 = 512 * qc
        # per half: [128 q, 4 qsb x 65] ctx accumulator in ONE psum bank.
        # Zero-region/group semantics: start=True only on the very first
        # matmul into the bank (zeroes the whole 2KB region; sibling qsb
        # slices consume the pending-zero with start=False), stop=True only
        # on the very last.
        ctxs = [psum_ctx.tile([128, 512], dt.float32, name="ctx")
                for _ in range(2)]
        started = [False, False]
        pend2 = []

        def emit_pv(kbs, offs, ns, pts):
            for half in range(2):
                for (kb, off, n) in zip(kbs, offs, ns):
                    j0 = (512 - n) // 128
                    for j in range(j0, 4):
                        c0 = off + (j - j0) * 128
                        nc.tensor.matmul(
                            ctxs[half][:, j * 65:j * 65 + 65],
                            lhsT=pts[half][:, c0:c0 + 128],
                            rhs=vo_sb[kb][:, 2 * hp + half, :],
                            start=not started[half],
                            stop=(kb == nkb - 1 and j == 3),
                        )
                        started[half] = True

        for kb0 in range(0, nkb, 2):
            kbs = [kb for kb in (kb0, kb0 + 1) if kb < nkb]
            ns = [512 - max(0, kb * 128 - q0) for kb in kbs]
            offs = [0] + [ns[0]] * (len(kbs) - 1)
            pts = []
            for half in range(2):
                p0 = half * 64
                stp = psum_st.tile([128, 1024], dt.float32, name="stp")
                for kb, off, n in zip(kbs, offs, ns):
                    diag = kb >= 4 * qc
                    nc.tensor.matmul(
                        stp[:, off:off + n],
                        lhsT=kt_sb[hp][p0:p0 + 64, kb * 128:(kb + 1) * 128],
                        rhs=qt_sb[hp][p0:p0 + 64, q0 + 512 - n:q0 + 512],
                        start=True,
                        stop=not diag,
                        tile_position=(p0, 0) if PACK_HEADS else None,
                    )
                    if diag:  # add -1e9 above the diagonal (first 128 cols)
                        nc.tensor.matmul(
                            stp[:, off:off + 128],
                            lhsT=ident,
                            rhs=msk,
                            start=False,
                            stop=True,
                        )
                ntot = offs[-1] + ns[-1]
                pt = pt_pool.tile([128, 1024], dt.bfloat16, name="pt")
                nc.scalar.activation(
                    out=pt[:, :ntot], in_=stp[:, :ntot], func=F.Exp,
                    scale=0.125)
                pts.append(pt)
            pend2.append((kbs, offs, ns, pts))
            if len(pend2) > 2:
                emit_pv(*pend2.pop(0))
        for p in pend2:
            emit_pv(*p)
        # normalize: per-partition scale by 1/denominator, then xbar-transpose
        # each [128 q, 128 d]-pair tile into cxtq (d on partitions).
        nqs = [nq_pool.tile([128, 128], dt.bfloat16, name="nq")
               for _ in range(4)]
        for half in range(2):
            ctx_t = ctxs[half]
            # [128, 4] AP of the 4 denominator columns (stride 65, offset 64)
            dens = ctx_t[:, 0:260].rearrange("p (j c) -> p c j", c=65)[:, 64, :]
            recip = small.tile([128, 4], dt.float32, name="recip")
            nc.vector.reciprocal(out=recip, in_=dens)
            for j in range(4):
                nc.vector.tensor_scalar_mul(
                    nqs[j][:, half * 64:half * 64 + 64],
                    ctx_t[:, j * 65:j * 65 + 64],
                    recip[:, j:j + 1],
                )
        for j in range(4):
            nc.sync.dma_start_transpose(
                out=cxtq_sb[(hp, 4 * qc + j)], in_=nqs[j])

    def emit_p4(sq_lo, sq_hi, final=False):
        for sq in range(sq_lo, sq_hi):
            for oc in range(2):
                if final:  # attention done: rotate over ALL freed banks
                    k = (sq * 2 + oc) % 3
                    if k == 0:
                        ps = psum_st.tile([128, 1024], dt.float32,
                                          name="stp")[:, 0:512]
                    elif k == 1:
                        ps = psum_ctx.tile([128, 512], dt.float32, name="ctx")
                    else:
                        ps = psum_mm.tile([128, 512], dt.float32, name="mmps")
                else:
                    ps = psum_mm.tile([128, 512], dt.float32, name="mmps")
                for dvt in range(4):
                    nc.tensor.matmul(
                        ps,
                        lhsT=cxtq_sb[(dvt, sq)],
                        rhs=ow_sb[dvt][:, oc * 512:(oc + 1) * 512],
                        start=(dvt == 0),
                        stop=(dvt == 3),
                    )
                ost = ost_pool.tile([128, 512], dt.float32, name="ost")
                nc.vector.tensor_copy(out=ost, in_=ps)
                nc.sync.dma_start(
                    out=outp.ap()[sq * 128:(sq + 1) * 128,
                                  oc * 512:(oc + 1) * 512],
                    in_=ost,
                )

    # ---- emission schedule: pipeline projections with qc=0 attention ----
    v_ranges = [range(0, 4), range(4, 8), range(8, 12), range(12, 16)]
    for dtile in range(4):
        emit_proj_dtile(wq_sb, qt_sb, dtile)
        emit_proj_dtile(wk_sb, kt_sb, dtile)
        for st in v_ranges[dtile]:
            emit_v(st)
        emit_attn_pair(0, dtile)
    for qc in range(1, NQC):
        for hp in range(4):
            emit_attn_pair(qc, hp)
            if hp == 1:
                emit_p4(4 * (qc - 1), 4 * qc)
    emit_p4(12, 16, final=True)

    return pools


def _build_nc():
    import concourse.tile as tile
    from concourse import bacc, mybir

    dt = mybir.dt
    nc = bacc.Bacc("TRN2", target_bir_lowering=False, debug=False,
                   num_devices=NCORES)
    xT = nc.dram_tensor("xt", [D, S], dt.bfloat16, kind="ExternalInput")
    wq = nc.dram_tensor("wq", [D, GD], dt.bfloat16, kind="ExternalInput")
    wk = nc.dram_tensor("wk", [D, GD], dt.bfloat16, kind="ExternalInput")
    wv = nc.dram_tensor("wv", [D, GD], dt.bfloat16, kind="ExternalInput")
    ow = nc.dram_tensor("ow", [GD, D], dt.bfloat16, kind="ExternalInput")
    outp = nc.dram_tensor("outp", [S, D], dt.float32, kind="ExternalOutput")

    with tile.TileContext(nc) as tc:
        pools = _build_body(tc, nc, mybir, xT, wq, wk, wv, ow, outp)
        pools.close()
    nc.compile()
    return nc


LAST_RESULTS = None


def kernel(batch, w_query, w_key, w_value, out_w, out_b):
    global LAST_RESULTS
    import os
    from concourse import bass_utils

    try:  # BASS_TRACE needs the axon NTFF hook; without it the run crashes
        from antenv.axon_hooks import get_axon_ntff_profile_hook  # noqa: F401
    except ImportError:
        os.environ.setdefault("BASS_NEVER_TRACE", "1")

    batch = np.asarray(batch, dtype=np.float32)
    w_query = np.asarray(w_query, dtype=np.float32)
    w_key = np.asarray(w_key, dtype=np.float32)
    w_value = np.asarray(w_value, dtype=np.float32)
    out_w = np.asarray(out_w, dtype=np.float32)
    out_b = np.asarray(out_b, dtype=np.float32)

    if "nc" not in _cache:
        _cache["nc"] = _build_nc()
    nc = _cache["nc"]

    xts = [np.ascontiguousarray(batch[b].T).astype(BF16) for b in range(B)]
    slc = [slice(g * GD, (g + 1) * GD) for g in range(2)]
    wqs = [np.ascontiguousarray(w_query[:, s]).astype(BF16) for s in slc]
    wks = [np.ascontiguousarray(w_key[:, s]).astype(BF16) for s in slc]
    wvs = [np.ascontiguousarray(w_value[:, s]).astype(BF16) for s in slc]
    ows = [np.ascontiguousarray(out_w[s, :]).astype(BF16) for s in slc]
    in_maps = []
    for c in range(NCORES):
        b, g = divmod(c, 2)
        in_maps.append({
            "xt": xts[b], "wq": wqs[g], "wk": wks[g],
            "wv": wvs[g], "ow": ows[g],
        })

    res = bass_utils.run_bass_kernel_spmd(
        nc, in_maps, core_ids=list(range(NCORES)),
    )
    LAST_RESULTS = res

    out = np.empty((B, S, D), np.float32)
    for b in range(B):
        out[b] = res.results[2 * b]["outp"] + res.results[2 * b + 1]["outp"] \
            + out_b[None, :]
    return out


# revision 9
# speedup vs baseline: 1.1402x; 1.1382x over previous
"""Multi-head causal attention (B=4, S=2048, D=1024, 16 heads) on 8 TRN2 cores.

Sharding: core c -> (batch b = c//2, head-group g = c%2). Each core computes
8 heads of one batch element end-to-end (QKV proj, causal softmax attention,
out-proj rows for its head slice). Host sums the two head-group partials per
batch and adds the output bias.

Per-core pipeline (all matmuls contraction-on-partitions, bf16 in / f32 psum):
  QT/KT[dtile] = (x @ w)^T   [128p = 2 heads x 64, S]
  Vones[kb]    = [V | 1]     [128p = k, h, 65]
  attention per (512-wide q-chunk, head-pair); the pair's score matmuls are
  packed into PE row groups via tile_position; two k-blocks share one
  [128,1024] score psum so each exp covers ~1024 cols; PV matmuls trail two
  iterations behind so PE never waits on the exp:
    ST[k,q] = KT.T @ QT; PT = exp(ST/8) bf16; tri-mask on diagonal 128 cols
    ctx[q 128, h, 65] += PT.T @ [V_h | 1]  (PT slice is the lhsT: output has
      q on partitions -> full 128-row PE output, ~half the PV stream cols of
      the [65, q] orientation; col 64 = softmax denominator)
    nq[q, d] = ctx * (1/denominator)  per-partition scale on DVE
    cxtq[d, q] = DMA-xbar-transpose(nq)  per (head-pair, 128-q tile)
  out[seq128, 512] = cxtq.T @ ow, streamed to DRAM per q-chunk; the final
  q-chunk's out-proj borrows the freed score psum banks.
"""

import numpy as np
import ml_dtypes

B, S, D = 4, 2048, 1024
H_TOT = 16
HD = 64
NCORES = 8
GH = 8          # heads per core
GD = GH * HD    # 512: dout slice per core
NKB = S // 128  # 16 k-blocks
NQC = S // 512  # 4 q-chunks
BF16 = ml_dtypes.bfloat16

PACK_HEADS = True   # pack 2 heads' score matmuls into PE row groups

_cache = {}


def _build_body(tc, nc, mybir, xT, wq, wk, wv, ow, outp):
    from concourse.masks import make_identity, make_lower_triangular
    import contextlib

    dt = mybir.dt
    F = mybir.ActivationFunctionType

    pools = contextlib.ExitStack()
    tc_pool = lambda **kw: pools.enter_context(tc.tile_pool(**kw))

    singles = tc_pool(name="singles", bufs=1)
    pt_pool = tc_pool(name="pt", bufs=8)
    small = tc_pool(name="small", bufs=4)
    nq_pool = tc_pool(name="nq", bufs=8)
    ost_pool = tc_pool(name="ost", bufs=5)
    psum_st = tc_pool(name="psum_st", bufs=2, space="PSUM")
    psum_ctx = tc_pool(name="psum_ctx", bufs=2, space="PSUM")
    psum_fl = tc_pool(name="psum_fl", bufs=2, space="PSUM")

    # ---- persistent SBUF tensors (split per producer/consumer region so the
    # dependency tracker never over-serializes) ----
    xT_sb = [singles.tile([128, S], dt.bfloat16, name=f"xt{t}")
             for t in range(8)]
    wq_sb = [singles.tile([128, GD], dt.bfloat16, name=f"wq{t}")
             for t in range(8)]
    wk_sb = [singles.tile([128, GD], dt.bfloat16, name=f"wk{t}")
             for t in range(8)]
    wv_sb = [singles.tile([128, GD], dt.bfloat16, name=f"wv{t}")
             for t in range(8)]
    ow_sb = [singles.tile([128, D], dt.bfloat16, name=f"ow{t}")
             for t in range(4)]
    qt_sb = [singles.tile([128, S], dt.bfloat16, name=f"qt{t}")
             for t in range(4)]                              # 2 heads / dtile
    kt_sb = [singles.tile([128, S], dt.bfloat16, name=f"kt{t}")
             for t in range(4)]
    vo_sb = [singles.tile([128, GH, 65], dt.bfloat16, name=f"vo{t}")
             for t in range(NKB)]                            # [V_h | ones]
    # normalized ctx^T tiles [128 = 2 heads x 64 d, 128 q] per (pair, seq128)
    cxtq_sb = {(hp, sq): singles.tile([128, 128], dt.bfloat16,
                                      name=f"cq{hp}_{sq}")
               for hp in range(4) for sq in range(NKB)}
    # causal mask folded into the score matmul: one extra accumulating
    # matmul per diagonal 128-col block adds -1e9 where k > q, so the exp
    # zeroes them and the DVE is off the exp->PV critical path entirely.
    ident = singles.tile([128, 128], dt.bfloat16)
    msk = singles.tile([128, 128], dt.bfloat16)              # -1e9 at k > q

    make_identity(nc, ident)
    make_lower_triangular(nc, msk, val=-1e9, diag=False)
    for t in range(NKB):
        nc.vector.memset(vo_sb[t][:, :, 64:65], 1.0)

    # ---- input DMAs (split per 128-row tile; first matmuls need wq t0 + xT t0) ----
    xT_r = xT.ap().rearrange("(t p) s -> p t s", p=128)
    wq_r = wq.ap().rearrange("(t p) n -> p t n", p=128)
    wk_r = wk.ap().rearrange("(t p) n -> p t n", p=128)
    wv_r = wv.ap().rearrange("(t p) n -> p t n", p=128)
    ow_r = ow.ap().rearrange("(t p) n -> p t n", p=128)
    DIN_ORDER = list(range(8))
    for t in range(8):
        nc.sync.dma_start(out=xT_sb[t], in_=xT_r[:, t, :])
        nc.sync.dma_start(out=wq_sb[t], in_=wq_r[:, t, :])
        if t % 2 == 1:
            nc.sync.dma_start(out=wk_sb[t // 2], in_=wk_r[:, t // 2, :])
    for t in range(4, 8):
        nc.sync.dma_start(out=wk_sb[t], in_=wk_r[:, t, :])
    for t in range(8):
        nc.sync.dma_start(out=wv_sb[t], in_=wv_r[:, t, :])
    for t in range(4):
        nc.sync.dma_start(out=ow_sb[t], in_=ow_r[:, t, :])

    # ================= filler units (PE work with no ACT dependency) ======
    # Each unit is a single [128, 512]-output psum bank accumulated over its
    # full contraction + one DVE drain copy. Units are interleaved between
    # attention iterations so the PE stays busy while the ACT engine (the
    # per-iteration bottleneck: 2 exps ~ 2us vs ~1.3us of PE work) churns.
    W_TILES = {"q": wq_sb, "k": wk_sb}
    T_TILES = {"q": qt_sb, "k": kt_sb}

    def emit_unit_fn(key):
        kind = key[0]
        if kind in ("q", "k"):
            _, hp, c = key
            ps = psum_fl.tile([128, 512], dt.float32, name="flps")
            for din in range(8):
                nc.tensor.matmul(
                    ps,
                    lhsT=W_TILES[kind][din][:, hp * 128:(hp + 1) * 128],
                    rhs=xT_sb[din][:, c * 512:(c + 1) * 512],
                    start=(din == 0),
                    stop=(din == 7),
                )
            nc.vector.tensor_copy(
                out=T_TILES[kind][hp][:, c * 512:(c + 1) * 512], in_=ps)
        elif kind == "v":
            _, st = key
            ps = psum_fl.tile([128, 512], dt.float32, name="flps")
            for din in range(8):
                nc.tensor.matmul(
                    ps,
                    lhsT=xT_sb[din][:, st * 128:(st + 1) * 128],
                    rhs=wv_sb[din],
                    start=(din == 0),
                    stop=(din == 7),
                )
            nc.vector.tensor_copy(
                out=vo_sb[st][:, :, 0:64],
                in_=ps.rearrange("p (h d) -> p h d", h=GH),
            )
        else:  # ("p4", sq, oc)
            _, sq, oc = key
            ps = psum_fl.tile([128, 512], dt.float32, name="flps")
            for dvt in range(4):
                nc.tensor.matmul(
                    ps,
                    lhsT=cxtq_sb[(dvt, sq)],
                    rhs=ow_sb[dvt][:, oc * 512:(oc + 1) * 512],
                    start=(dvt == 0),
                    stop=(dvt == 3),
                )
            ost = ost_pool.tile([128, 512], dt.float32, name="ost")
            nc.vector.tensor_copy(out=ost, in_=ps)
            nc.sync.dma_start(
                out=outp.ap()[sq * 128:(sq + 1) * 128,
                              oc * 512:(oc + 1) * 512],
                in_=ost,
            )

    UNIT_PE = {"q": 1707, "k": 1707, "v": 1707, "p4": 853}
    emitted = set()
    lazy_q = []
    clock = {"pe": 0.0, "act": 0.0}

    def emit_unit(key):
        if key in emitted:
            return
        emitted.add(key)
        emit_unit_fn(key)
        clock["pe"] += UNIT_PE[key[0]]

    def pop_lazy():
        while lazy_q:
            key = lazy_q[0]
            if key in emitted:
                lazy_q.pop(0)
                continue
            if clock["pe"] + UNIT_PE[key[0]] > clock["act"]:
                break
            emit_unit(lazy_q.pop(0))

    # ===================== attention block =================================
    def attn_block(qc, hp):
        """512-wide q chunk qc for heads h0=2*hp (PE rows 0:64) and
        h1=2*hp+1 (rows 64:128).

        Two k-blocks share one [128,1024] ST psum tile so each exp covers
        ~1024 cols; the causal mask is a -1e9 accumulating matmul on the
        diagonal blocks; PV (PT as lhsT, [V|1] as rhs, out [128 q, 65])
        trails two iterations so the PE never waits on the exp. Yields
        between k-block-pair iterations so the driver can interleave filler
        units while the ACT engine is the per-iteration bottleneck."""
        nkb = 4 * qc + 4
        q0 = 512 * qc
        ctxs = [psum_ctx.tile([128, 512], dt.float32, name="ctx")
                for _ in range(2)]
        started = [False, False]
        pend2 = []

        def emit_pv(kbs, offs, ns, pts):
            for half in range(2):
                for (kb, off, n) in zip(kbs, offs, ns):
                    j0 = (512 - n) // 128
                    for j in range(j0, 4):
                        c0 = off + (j - j0) * 128
                        nc.tensor.matmul(
                            ctxs[half][:, j * 65:j * 65 + 65],
                            lhsT=pts[half][:, c0:c0 + 128],
                            rhs=vo_sb[kb][:, 2 * hp + half, :],
                            start=not started[half],
                            stop=(kb == nkb - 1 and j == 3),
                        )
                        started[half] = True

        for kb0 in range(0, nkb, 2):
            kbs = [kb for kb in (kb0, kb0 + 1) if kb < nkb]
            ns = [512 - max(0, kb * 128 - q0) for kb in kbs]
            offs = [0] + [ns[0]] * (len(kbs) - 1)
            pts = []
            ndiag = 0
            for half in range(2):
                p0 = half * 64
                stp = psum_st.tile([128, 1024], dt.float32, name="stp")
                for kb, off, n in zip(kbs, offs, ns):
                    diag = kb >= 4 * qc
                    nc.tensor.matmul(
                        stp[:, off:off + n],
                        lhsT=kt_sb[hp][p0:p0 + 64, kb * 128:(kb + 1) * 128],
                        rhs=qt_sb[hp][p0:p0 + 64, q0 + 512 - n:q0 + 512],
                        start=True,
                        stop=not diag,
                        tile_position=(p0, 0) if PACK_HEADS else None,
                    )
                    if diag:  # add -1e9 above the diagonal (first 128 cols)
                        ndiag += 1
                        nc.tensor.matmul(
                            stp[:, off:off + 128],
                            lhsT=ident,
                            rhs=msk,
                            start=False,
                            stop=True,
                        )
                ntot = offs[-1] + ns[-1]
                pt = pt_pool.tile([128, 1024], dt.bfloat16, name="pt")
                nc.scalar.activation(
                    out=pt[:, :ntot], in_=stp[:, :ntot], func=F.Exp,
                    scale=0.125)
                pts.append(pt)
            pend2.append((kbs, offs, ns, pts))
            if len(pend2) > 2:
                emit_pv(*pend2.pop(0))
            ntot = offs[-1] + ns[-1]
            nqsb = sum(4 - (512 - n) // 128 for n in ns)
            clock["act"] += 2 * (ntot * 0.8333 + 190)
            clock["pe"] += (2 * ntot + 65 * 2 * nqsb + 128 * ndiag) * 0.4167
            yield
        for p in pend2:
            emit_pv(*p)
        # normalize: per-partition scale by 1/denominator, then xbar-transpose
        # each [128 q, 128 d]-pair tile into cxtq (d on partitions).
        nqs = [nq_pool.tile([128, 128], dt.bfloat16, name="nq")
               for _ in range(4)]
        for half in range(2):
            ctx_t = ctxs[half]
            # [128, 4] AP of the 4 denominator columns (stride 65, offset 64)
            dens = ctx_t[:, 0:260].rearrange("p (j c) -> p c j", c=65)[:, 64, :]
            recip = small.tile([128, 4], dt.float32, name="recip")
            nc.vector.reciprocal(out=recip, in_=dens)
            for j in range(4):
                nc.vector.tensor_scalar_mul(
                    nqs[j][:, half * 64:half * 64 + 64],
                    ctx_t[:, j * 65:j * 65 + 64],
                    recip[:, j:j + 1],
                )
        for j in range(4):
            nc.sync.dma_start_transpose(
                out=cxtq_sb[(hp, 4 * qc + j)], in_=nqs[j])

    # ===================== driver ==========================================
    # hp-outer: per head-pair run all q-chunks; QK projections for later
    # head-pairs, V tiles, and the out-proj of finished seq tiles fill the
    # PE while the ACT exp stream runs. Lazy units pace against a simple
    # ACT-vs-PE clock; deadline units are forced before the block needing
    # them.
    for qc in range(NQC):  # hp0 consumes V and its own QK chunks per qc
        lazy_q.append(("q", 0, qc))
        lazy_q.append(("k", 0, qc))
        lazy_q.extend(("v", st) for st in range(4 * qc, 4 * qc + 4))
    for hp in range(1, 4):
        for qc in range(NQC):
            lazy_q.append(("q", hp, qc))
            lazy_q.append(("k", hp, qc))

    for hp in range(4):
        for qc in range(NQC):
            emit_unit(("q", hp, qc))
            emit_unit(("k", hp, qc))
            for st in range(4 * qc, 4 * qc + 4):
                emit_unit(("v", st))
            if hp == 3 and qc > 0:  # prev q-chunk's out-proj now has all hps
                for sq in range(4 * (qc - 1), 4 * qc):
                    for oc in range(2):
                        lazy_q.append(("p4", sq, oc))
            for _ in attn_block(qc, hp):
                pop_lazy()
    for sq in range(NKB):
        for oc in range(2):
            emit_unit(("p4", sq, oc))

    return pools


def _build_nc():
    import concourse.tile as tile
    from concourse import bacc, mybir

    dt = mybir.dt
    nc = bacc.Bacc("TRN2", target_bir_lowering=False, debug=False,
                   num_devices=NCORES)
    xT = nc.dram_tensor("xt", [D, S], dt.bfloat16, kind="ExternalInput")
    wq = nc.dram_tensor("wq", [D, GD], dt.bfloat16, kind="ExternalInput")
    wk = nc.dram_tensor("wk", [D, GD], dt.bfloat16, kind="ExternalInput")
    wv = nc.dram_tensor("wv", [D, GD], dt.bfloat16, kind="ExternalInput")
    ow = nc.dram_tensor("ow", [GD, D], dt.bfloat16, kind="ExternalInput")
    outp = nc.dram_tensor("outp", [S, D], dt.float32, kind="ExternalOutput")

    with tile.TileContext(nc) as tc:
        pools = _build_body(tc, nc, mybir, xT, wq, wk, wv, ow, outp)
        pools.close()
    nc.compile()
    return nc


LAST_RESULTS = None


def kernel(batch, w_query, w_key, w_value, out_w, out_b):
    global LAST_RESULTS
    import os
    from concourse import bass_utils

    try:  # BASS_TRACE needs the axon NTFF hook; without it the run crashes
        from antenv.axon_hooks import get_axon_ntff_profile_hook  # noqa: F401
    except ImportError:
        os.environ.setdefault("BASS_NEVER_TRACE", "1")

    batch = np.asarray(batch, dtype=np.float32)
    w_query = np.asarray(w_query, dtype=np.float32)
    w_key = np.asarray(w_key, dtype=np.float32)
    w_value = np.asarray(w_value, dtype=np.float32)
    out_w = np.asarray(out_w, dtype=np.float32)
    out_b = np.asarray(out_b, dtype=np.float32)

    if "nc" not in _cache:
        _cache["nc"] = _build_nc()
    nc = _cache["nc"]

    xts = [np.ascontiguousarray(batch[b].T).astype(BF16) for b in range(B)]
    slc = [slice(g * GD, (g + 1) * GD) for g in range(2)]
    wqs = [np.ascontiguousarray(w_query[:, s]).astype(BF16) for s in slc]
    wks = [np.ascontiguousarray(w_key[:, s]).astype(BF16) for s in slc]
    wvs = [np.ascontiguousarray(w_value[:, s]).astype(BF16) for s in slc]
    ows = [np.ascontiguousarray(out_w[s, :]).astype(BF16) for s in slc]
    in_maps = []
    for c in range(NCORES):
        b, g = divmod(c, 2)
        in_maps.append({
            "xt": xts[b], "wq": wqs[g], "wk": wks[g],
            "wv": wvs[g], "ow": ows[g],
        })

    res = bass_utils.run_bass_kernel_spmd(
        nc, in_maps, core_ids=list(range(NCORES)),
    )
    LAST_RESULTS = res

    out = np.empty((B, S, D), np.float32)
    for b in range(B):
        out[b] = res.results[2 * b]["outp"] + res.results[2 * b + 1]["outp"] \
            + out_b[None, :]
    return out


# revision 14
# speedup vs baseline: 1.1934x; 1.0466x over previous
"""Multi-head causal attention (B=4, S=2048, D=1024, 16 heads) on 8 TRN2 cores.

Sharding: core c -> (batch b = c//2, head-group g = c%2). Each core computes
8 heads of one batch element end-to-end (QKV proj, causal softmax attention,
out-proj rows for its head slice). Host sums the two head-group partials per
batch and adds the output bias.

Per-core pipeline (all matmuls contraction-on-partitions, bf16 in / f32 psum):
  QT/KT[dtile] = (x @ w)^T   [128p = 2 heads x 64, S]
  Vones[kb]    = [V | 1]     [128p = k, h, 65]
  attention per (512-wide q-chunk, head-pair); the pair's score matmuls are
  packed into PE row groups via tile_position; two k-blocks share one
  [128,1024] score psum so each exp covers ~1024 cols; PV matmuls trail two
  iterations behind so PE never waits on the exp:
    ST[k,q] = KT.T @ QT; PT = exp(ST/8) bf16; tri-mask on diagonal 128 cols
    ctx[q 128, h, 65] += PT.T @ [V_h | 1]  (PT slice is the lhsT: output has
      q on partitions -> full 128-row PE output, ~half the PV stream cols of
      the [65, q] orientation; col 64 = softmax denominator)
    nq[q, d] = ctx * (1/denominator)  per-partition scale on DVE
    cxtq[d, q] = DMA-xbar-transpose(nq)  per (head-pair, 128-q tile)
  out[seq128, 512] = cxtq.T @ ow, streamed to DRAM per q-chunk; the final
  q-chunk's out-proj borrows the freed score psum banks.
"""

import numpy as np
import ml_dtypes

B, S, D = 4, 2048, 1024
H_TOT = 16
HD = 64
NCORES = 8
GH = 8          # heads per core
GD = GH * HD    # 512: dout slice per core
NKB = S // 128  # 16 k-blocks
NQC = S // 512  # 4 q-chunks
BF16 = ml_dtypes.bfloat16

PACK_HEADS = True   # pack 2 heads' score matmuls into PE row groups

_cache = {}


def _build_body(tc, nc, mybir, xT, wq, wk, wv, ow, outp):
    from concourse.masks import make_upper_triangular
    import contextlib

    dt = mybir.dt
    F = mybir.ActivationFunctionType

    pools = contextlib.ExitStack()
    tc_pool = lambda **kw: pools.enter_context(tc.tile_pool(**kw))

    singles = tc_pool(name="singles", bufs=1)
    pt_pool = tc_pool(name="pt", bufs=8)
    small = tc_pool(name="small", bufs=4)
    nq_pool = tc_pool(name="nq", bufs=8)
    ost_pool = tc_pool(name="ost", bufs=5)
    psum_st = tc_pool(name="psum_st", bufs=2, space="PSUM")
    psum_ctx = tc_pool(name="psum_ctx", bufs=2, space="PSUM")
    psum_fl = tc_pool(name="psum_fl", bufs=2, space="PSUM")

    # ---- persistent SBUF tensors (split per producer/consumer region so the
    # dependency tracker never over-serializes) ----
    xT_sb = [singles.tile([128, S], dt.bfloat16, name=f"xt{t}")
             for t in range(8)]
    wq_sb = [singles.tile([128, GD], dt.bfloat16, name=f"wq{t}")
             for t in range(8)]
    wk_sb = [singles.tile([128, GD], dt.bfloat16, name=f"wk{t}")
             for t in range(8)]
    wv_sb = [singles.tile([128, GD], dt.bfloat16, name=f"wv{t}")
             for t in range(8)]
    ow_sb = [singles.tile([128, D], dt.bfloat16, name=f"ow{t}")
             for t in range(4)]
    qt_sb = [singles.tile([128, S], dt.bfloat16, name=f"qt{t}")
             for t in range(4)]                              # 2 heads / dtile
    kt_sb = [singles.tile([128, S], dt.bfloat16, name=f"kt{t}")
             for t in range(4)]
    vo_sb = [singles.tile([128, GH, 65], dt.bfloat16, name=f"vo{t}")
             for t in range(NKB)]                            # [V_h | ones]
    # normalized ctx^T tiles [128 = 2 heads x 64 d, 128 q] per (pair, seq128)
    cxtq_sb = {(hp, sq): singles.tile([128, 128], dt.bfloat16,
                                      name=f"cq{hp}_{sq}")
               for hp in range(4) for sq in range(NKB)}
    tri = singles.tile([128, 128], dt.bfloat16)              # keep k<=q

    make_upper_triangular(nc, tri, val=1.0, diag=True)
    for t in range(NKB):
        nc.vector.memset(vo_sb[t][:, :, 64:65], 1.0)

    # ---- input DMAs, ordered by first consumption. xT is split into 512-col
    # chunks so the first QK projection units (which only read q/k columns
    # 0:512) start after ~3MB instead of the full 5MB of input. ----
    xT_r = xT.ap().rearrange("(t p) s -> p t s", p=128)
    wq_r = wq.ap().rearrange("(t p) n -> p t n", p=128)
    wk_r = wk.ap().rearrange("(t p) n -> p t n", p=128)
    wv_r = wv.ap().rearrange("(t p) n -> p t n", p=128)
    ow_r = ow.ap().rearrange("(t p) n -> p t n", p=128)
    for t in range(8):
        nc.sync.dma_start(out=wq_sb[t], in_=wq_r[:, t, :])
        nc.sync.dma_start(out=wk_sb[t], in_=wk_r[:, t, :])
        nc.sync.dma_start(out=xT_sb[t][:, 0:512], in_=xT_r[:, t, 0:512])
    for t in range(8):
        nc.sync.dma_start(out=wv_sb[t], in_=wv_r[:, t, :])
    for c in range(1, 4):
        for t in range(8):
            nc.sync.dma_start(out=xT_sb[t][:, c * 512:(c + 1) * 512],
                              in_=xT_r[:, t, c * 512:(c + 1) * 512])
    for t in range(4):
        nc.sync.dma_start(out=ow_sb[t], in_=ow_r[:, t, :])

    # ================= filler units (PE work with no ACT dependency) ======
    # Each unit is a single [128, 512]-output psum bank accumulated over its
    # full contraction + one DVE drain copy. Units are interleaved between
    # attention iterations so the PE stays busy while the ACT engine (the
    # per-iteration bottleneck: 2 exps ~ 2us vs ~1.3us of PE work) churns.
    W_TILES = {"q": wq_sb, "k": wk_sb}
    T_TILES = {"q": qt_sb, "k": kt_sb}

    def emit_unit_fn(key):
        kind = key[0]
        if kind in ("q", "k"):
            _, hp, c = key
            ps = psum_fl.tile([128, 512], dt.float32, name="flps")
            for din in range(8):
                nc.tensor.matmul(
                    ps,
                    lhsT=W_TILES[kind][din][:, hp * 128:(hp + 1) * 128],
                    rhs=xT_sb[din][:, c * 512:(c + 1) * 512],
                    start=(din == 0),
                    stop=(din == 7),
                )
            nc.vector.tensor_copy(
                out=T_TILES[kind][hp][:, c * 512:(c + 1) * 512], in_=ps)
        elif kind == "v":
            _, st = key
            ps = psum_fl.tile([128, 512], dt.float32, name="flps")
            for din in range(8):
                nc.tensor.matmul(
                    ps,
                    lhsT=xT_sb[din][:, st * 128:(st + 1) * 128],
                    rhs=wv_sb[din],
                    start=(din == 0),
                    stop=(din == 7),
                )
            nc.vector.tensor_copy(
                out=vo_sb[st][:, :, 0:64],
                in_=ps.rearrange("p (h d) -> p h d", h=GH),
            )
        else:  # ("p4", sq, oc)
            _, sq, oc = key
            ps = psum_fl.tile([128, 512], dt.float32, name="flps")
            for dvt in range(4):
                nc.tensor.matmul(
                    ps,
                    lhsT=cxtq_sb[(dvt, sq)],
                    rhs=ow_sb[dvt][:, oc * 512:(oc + 1) * 512],
                    start=(dvt == 0),
                    stop=(dvt == 3),
                )
            ost = ost_pool.tile([128, 512], dt.float32, name="ost")
            nc.vector.tensor_copy(out=ost, in_=ps)
            nc.sync.dma_start(
                out=outp.ap()[sq * 128:(sq + 1) * 128,
                              oc * 512:(oc + 1) * 512],
                in_=ost,
            )

    UNIT_PE = {"q": 1707, "k": 1707, "v": 1707, "p4": 853}
    emitted = set()
    lazy_q = []
    clock = {"pe": 0.0, "act": 0.0}

    def emit_unit(key):
        if key in emitted:
            return
        emitted.add(key)
        emit_unit_fn(key)
        clock["pe"] += UNIT_PE[key[0]]

    def pop_lazy():
        while lazy_q:
            key = lazy_q[0]
            if key in emitted:
                lazy_q.pop(0)
                continue
            if clock["pe"] + UNIT_PE[key[0]] > clock["act"]:
                break
            emit_unit(lazy_q.pop(0))

    # ===================== attention block =================================
    def attn_block(qc, hp):
        """512-wide q chunk qc for heads h0=2*hp (PE rows 0:64) and
        h1=2*hp+1 (rows 64:128).

        Two k-blocks share one [128,1024] ST psum tile so each exp covers
        ~1024 cols; the causal mask is a -1e9 accumulating matmul on the
        diagonal blocks; PV (PT as lhsT, [V|1] as rhs, out [128 q, 65])
        trails two iterations so the PE never waits on the exp. Yields
        between k-block-pair iterations so the driver can interleave filler
        units while the ACT engine is the per-iteration bottleneck."""
        nkb = 4 * qc + 4
        q0 = 512 * qc
        ctxs = [psum_ctx.tile([128, 512], dt.float32, name="ctx")
                for _ in range(2)]
        started = [False, False]
        pend2 = []

        def emit_pv(kbs, offs, ns, pts):
            for half in range(2):
                for (kb, off, n) in zip(kbs, offs, ns):
                    j0 = (512 - n) // 128
                    for j in range(j0, 4):
                        c0 = off + (j - j0) * 128
                        nc.tensor.matmul(
                            ctxs[half][:, j * 65:j * 65 + 65],
                            lhsT=pts[half][:, c0:c0 + 128],
                            rhs=vo_sb[kb][:, 2 * hp + half, :],
                            start=not started[half],
                            stop=(kb == nkb - 1 and j == 3),
                        )
                        started[half] = True

        for kb0 in range(0, nkb, 2):
            kbs = [kb for kb in (kb0, kb0 + 1) if kb < nkb]
            ns = [512 - max(0, kb * 128 - q0) for kb in kbs]
            offs = [0] + [ns[0]] * (len(kbs) - 1)
            pts = []
            for half in range(2):
                p0 = half * 64
                stp = psum_st.tile([128, 1024], dt.float32, name="stp")
                for kb, off, n in zip(kbs, offs, ns):
                    nc.tensor.matmul(
                        stp[:, off:off + n],
                        lhsT=kt_sb[hp][p0:p0 + 64, kb * 128:(kb + 1) * 128],
                        rhs=qt_sb[hp][p0:p0 + 64, q0 + 512 - n:q0 + 512],
                        start=True,
                        stop=True,
                        tile_position=(p0, 0) if PACK_HEADS else None,
                    )
                ntot = offs[-1] + ns[-1]
                pt = pt_pool.tile([128, 1024], dt.bfloat16, name="pt")
                nc.scalar.activation(
                    out=pt[:, :ntot], in_=stp[:, :ntot], func=F.Exp,
                    scale=0.125)
                for kb, off in zip(kbs, offs):
                    if kb >= 4 * qc:  # diagonal: mask first 128 cols
                        nc.vector.tensor_mul(
                            pt[:, off:off + 128], pt[:, off:off + 128], tri)
                pts.append(pt)
            pend2.append((kbs, offs, ns, pts))
            if len(pend2) > 2:
                emit_pv(*pend2.pop(0))
            ntot = offs[-1] + ns[-1]
            nqsb = sum(4 - (512 - n) // 128 for n in ns)
            clock["act"] += 2 * (ntot * 0.8333 + 190)
            clock["pe"] += (2 * ntot + 65 * 2 * nqsb) * 0.4167
            yield
        for p in pend2:
            emit_pv(*p)
        # normalize: per-partition scale by 1/denominator, then xbar-transpose
        # each [128 q, 128 d]-pair tile into cxtq (d on partitions).
        nqs = [nq_pool.tile([128, 128], dt.bfloat16, name="nq")
               for _ in range(4)]
        for half in range(2):
            ctx_t = ctxs[half]
            # [128, 4] AP of the 4 denominator columns (stride 65, offset 64)
            dens = ctx_t[:, 0:260].rearrange("p (j c) -> p c j", c=65)[:, 64, :]
            recip = small.tile([128, 4], dt.float32, name="recip")
            nc.vector.reciprocal(out=recip, in_=dens)
            for j in range(4):
                nc.vector.tensor_scalar_mul(
                    nqs[j][:, half * 64:half * 64 + 64],
                    ctx_t[:, j * 65:j * 65 + 64],
                    recip[:, j:j + 1],
                )
        for j in range(4):
            nc.sync.dma_start_transpose(
                out=cxtq_sb[(hp, 4 * qc + j)], in_=nqs[j])

    # ===================== driver ==========================================
    # Anti-diagonal wavefront over (head-pair, q-chunk): each head-pair's
    # q-chunks still run in order (kt accumulates per chunk), but head-pairs
    # are staggered so sq groups finish progressively and the out-proj fills
    # the late ACT-bound windows instead of piling into a tail. QK/V units
    # for later blocks fill the PE between attention iterations, paced
    # against a simple ACT-vs-PE clock; deadline units are forced.
    ORDER = [(0, 0), (0, 1), (1, 0), (0, 2), (1, 1), (2, 0), (0, 3), (1, 2),
             (2, 1), (3, 0), (1, 3), (2, 2), (3, 1), (2, 3), (3, 2), (3, 3)]
    vseen = set()
    for hp, qc in ORDER:
        lazy_q.append(("q", hp, qc))
        lazy_q.append(("k", hp, qc))
        if qc not in vseen:
            vseen.add(qc)
            lazy_q.extend(("v", st) for st in range(4 * qc, 4 * qc + 4))

    for hp, qc in ORDER:
        emit_unit(("q", hp, qc))
        emit_unit(("k", hp, qc))
        for st in range(4 * qc, 4 * qc + 4):
            emit_unit(("v", st))
        for _ in attn_block(qc, hp):
            pop_lazy()
        if hp == 3:  # sq group qc now has all head-pairs' ctx
            for sq in range(4 * qc, 4 * qc + 4):
                for oc in range(2):
                    lazy_q.append(("p4", sq, oc))
    for sq in range(NKB):
        for oc in range(2):
            emit_unit(("p4", sq, oc))

    return pools


def _build_nc():
    import concourse.tile as tile
    from concourse import bacc, mybir

    dt = mybir.dt
    nc = bacc.Bacc("TRN2", target_bir_lowering=False, debug=False,
                   num_devices=NCORES)
    xT = nc.dram_tensor("xt", [D, S], dt.bfloat16, kind="ExternalInput")
    wq = nc.dram_tensor("wq", [D, GD], dt.bfloat16, kind="ExternalInput")
    wk = nc.dram_tensor("wk", [D, GD], dt.bfloat16, kind="ExternalInput")
    wv = nc.dram_tensor("wv", [D, GD], dt.bfloat16, kind="ExternalInput")
    ow = nc.dram_tensor("ow", [GD, D], dt.bfloat16, kind="ExternalInput")
    outp = nc.dram_tensor("outp", [S, D], dt.float32, kind="ExternalOutput")

    with tile.TileContext(nc) as tc:
        pools = _build_body(tc, nc, mybir, xT, wq, wk, wv, ow, outp)
        pools.close()
    nc.compile()
    return nc


LAST_RESULTS = None


def kernel(batch, w_query, w_key, w_value, out_w, out_b):
    global LAST_RESULTS
    import os
    from concourse import bass_utils

    try:  # BASS_TRACE needs the axon NTFF hook; without it the run crashes
        from antenv.axon_hooks import get_axon_ntff_profile_hook  # noqa: F401
    except ImportError:
        os.environ.setdefault("BASS_NEVER_TRACE", "1")

    batch = np.asarray(batch, dtype=np.float32)
    w_query = np.asarray(w_query, dtype=np.float32)
    w_key = np.asarray(w_key, dtype=np.float32)
    w_value = np.asarray(w_value, dtype=np.float32)
    out_w = np.asarray(out_w, dtype=np.float32)
    out_b = np.asarray(out_b, dtype=np.float32)

    if "nc" not in _cache:
        _cache["nc"] = _build_nc()
    nc = _cache["nc"]

    xts = [np.ascontiguousarray(batch[b].T).astype(BF16) for b in range(B)]
    slc = [slice(g * GD, (g + 1) * GD) for g in range(2)]
    wqs = [np.ascontiguousarray(w_query[:, s]).astype(BF16) for s in slc]
    wks = [np.ascontiguousarray(w_key[:, s]).astype(BF16) for s in slc]
    wvs = [np.ascontiguousarray(w_value[:, s]).astype(BF16) for s in slc]
    ows = [np.ascontiguousarray(out_w[s, :]).astype(BF16) for s in slc]
    in_maps = []
    for c in range(NCORES):
        b, g = divmod(c, 2)
        in_maps.append({
            "xt": xts[b], "wq": wqs[g], "wk": wks[g],
            "wv": wvs[g], "ow": ows[g],
        })

    res = bass_utils.run_bass_kernel_spmd(
        nc, in_maps, core_ids=list(range(NCORES)),
    )
    LAST_RESULTS = res

    out = np.empty((B, S, D), np.float32)
    for b in range(B):
        out[b] = res.results[2 * b]["outp"] + res.results[2 * b + 1]["outp"] \
            + out_b[None, :]
    return out
